# revision 24
# baseline (speedup 1.0000x reference)
"""Trainium2 Bass kernel for nn_Classifier_64587718197982 (spiking CNN).

Network (per reference):
  3x [conv3x3(C=128, pad=1, no bias) -> TDBN (batchnorm over T,B,H,W) -> LIF]
  -> mean over (H,W) -> mean over T -> FC(128->10)

Sharding: data-parallel over batch B=32 across 8 NeuronCores (4 images/core).
TDBN mean/var become a tiny [128,2] AllReduce per layer.

Per-core design (all activations SBUF-resident, no DRAM round-trips):
  conv L1: fp32r matmuls (full PE rate at free=512) over zero-padded
    [C,34,36] f32 input tiles; 9 full-window tap matmuls accumulate into
    8 PSUM banks (2 halves x 4 images per timestep), tap-outer ordering
    for stationary-weight reuse.
  conv L2/L3: fp8e4 DoubleRow matmuls (2x PE rate) over +-1 sign-spike
    tiles padded to row stride 48 so vertical tap pairs have AP step 48
    (DoubleRow requires pair-dim step % 16 == 0). 9 taps -> 6 pair
    matmuls (3 dummy taps, zero weights, slop row), x2 for hi/lo fp8
    weight splitting. Weights are pre-scaled by 64 host-side so the lo
    residuals stay in e4m3's normal range; BN is exactly invariant to
    any positive per-layer weight scale. Spikes stored as sign {-1,+1}
    with pad -1; BN absorbs the {0,1}<->{-1,1} affine exactly (only the
    variance scales: eps' = 4*eps).
  y activations: f32, SBUF-resident in a 36-slot rolling pool shared by
    all three layers (layer l images map to slot (BASE_l + img) % 36;
    BASE = 0/32/28 so each conv write lands on a slot whose reader
    already ran).
  TDBN: bn_stats on every other PSUM tile -> bn_aggr -> AllReduce of
    [mean, var+mean^2] -> sd = sqrt(var_s+eps'), u-space constants
    dp = beta*sd/gamma - mean_s, thp = theta*sd/gamma.
  LIF (u-space, u = mem/s', f32): per (t,b):
    gate = 0.25*(u<=thp)       [Pool]
    u *= gate                   [DVE]
    u = (y + dp) + u            [DVE scalar_tensor_tensor]
    spike = Sign(u - thp)       [Act] -> fp8 padded tile (L1/L2)
                                      -> fp16 + accum_out poolbuf (L3)
  conv l+1 of timestep t is emitted right after LIF l of timestep t so
  PE overlaps the elementwise engines.
  head: feat = (sum_t accum)/2 + T*HW/2; fc via broadcast mult + GpSimd
  partition all-reduce; logits = feat@fcW.T/8192 + b.
"""
import numpy as np
import ml_dtypes
from contextlib import ExitStack

import concourse.bass as bass
import concourse.mybir as mybir
import concourse.tile as tile
from concourse import bass_isa
from concourse import bacc
from concourse.bass_utils import run_bass_kernel_spmd

F32 = mybir.dt.float32
F32R = mybir.dt.float32r
FP16 = mybir.dt.float16
FP8 = mybir.dt.float8e4
AF = mybir.ActivationFunctionType
ALU = mybir.AluOpType
DR = mybir.MatmulPerfMode.DoubleRow

T, B, C, H, W = 8, 32, 128, 32, 32
NCORES = 8
BL = B // NCORES          # images per core per timestep
HW = H * W                # 1024
NIMG = T * BL             # 32 images per core
NHALF = 2
RH = H // NHALF           # 16 rows per half-tile
# spike tiles: padded rows 34 (+1 slop), row stride 48 (DoubleRow step)
SHP, SWP, SHPS = 34, 48, 35
SPADN = SHPS * SWP        # 1680
# L1 input tiles: padded rows 34, row stride 34 (no step constraint)
XWP = 34
XPADN = 34 * XWP          # 1156
NXBUF = 4
DECAY = 0.25
THRESH = 0.5
BN_EPS = 1e-5
W8SCALE = 64.0            # host pre-scale of L2/L3 weights (BN-invariant)
POOL_N = float(T * HW)    # pooling divisor 8192
NSLOT = 34                # rolling f32 y slots
YBASE = {1: 0, 2: 32, 3: 30}
STATS_STRIDE = 1          # bn_stats on every PSUM tile (exact batch stats)

SIM1 = False              # single-core TimelineSim variant (no collectives)
DEBUG = False             # add per-layer debug outputs
REPS = 1                  # replicate the whole body inside the NEFF (timing)

TAPS = [(dy, dx) for dy in range(3) for dx in range(3)]
# 6 geometric DoubleRow pairs (vertical, step 48); x2 terms (hi, lo)
GPAIRS = [((0, dx), (1, dx)) for dx in range(3)] + \
         [((2, dx), (3, dx)) for dx in range(3)]
NPAIR = 2 * len(GPAIRS)   # 12 pair-slots in the packed weight tensor


def build():
    nc = bacc.Bacc("TRN2", target_bir_lowering=False, debug=False,
                   num_devices=1 if SIM1 else NCORES)

    # --- I/O ---
    x_d = nc.dram_tensor("x", [T, BL, C, XPADN], F32, kind="ExternalInput")
    w1_d = nc.dram_tensor("w1", [C, 9, C], F32, kind="ExternalInput")
    w2_d = nc.dram_tensor("w2", [C, NPAIR, 2, C], FP8, kind="ExternalInput")
    w3_d = nc.dram_tensor("w3", [C, NPAIR, 2, C], FP8, kind="ExternalInput")
    bn_d = {}
    for l in (1, 2, 3):
        bn_d[(l, "w")] = nc.dram_tensor(f"bnw{l}", [C, 1], F32,
                                        kind="ExternalInput")
        bn_d[(l, "b")] = nc.dram_tensor(f"bnb{l}", [C, 1], F32,
                                        kind="ExternalInput")
    fcw_d = nc.dram_tensor("fcw", [C, 10], F32, kind="ExternalInput")
    fcb_d = nc.dram_tensor("fcb", [1, 10], F32, kind="ExternalInput")
    out_d = nc.dram_tensor("out", [1, BL * 10], F32, kind="ExternalOutput")
    dbg = {}
    if DEBUG:
        for l in (1, 2, 3):
            dbg[f"y{l}"] = nc.dram_tensor(f"dbg_y{l}", [C, HW], F32,
                                          kind="ExternalOutput")
            dbg[f"ccr{l}"] = nc.dram_tensor(f"dbg_ccr{l}", [C, 2], F32,
                                            kind="ExternalOutput")
            dbg[f"dp{l}"] = nc.dram_tensor(f"dbg_dp{l}", [C, 1], F32,
                                           kind="ExternalOutput")
            dbg[f"thp{l}"] = nc.dram_tensor(f"dbg_thp{l}", [C, 1], F32,
                                            kind="ExternalOutput")
        for l in (1, 2):
            dbg[f"spk{l}"] = nc.dram_tensor(f"dbg_spk{l}", [C, SPADN], FP8,
                                            kind="ExternalOutput")
        dbg["pool"] = nc.dram_tensor("dbg_pool", [C, T * BL], F32,
                                     kind="ExternalOutput")

    cc_bufs = {}
    for l in (1, 2, 3):
        cc_bufs[l] = (
            nc.dram_tensor(f"cc_in{l}", [C, 2], F32),
            nc.dram_tensor(f"cc_out{l}", [C, 2], F32, addr_space="Shared"),
        )

    with ExitStack() as ctx:
        tc = ctx.enter_context(tile.TileContext(nc))
        sb = ctx.enter_context(tc.tile_pool(name="sb", bufs=1))
        psum_pool = ctx.enter_context(
            tc.tile_pool(name="psum", bufs=1, space="PSUM"))

        # --- persistent tiles ---
        w1_sb = sb.tile([C, 9, C], F32R, name="w1")
        nc.sync.dma_start(out=w1_sb, in_=w1_d[:, :, :].bitcast(F32R))
        w2_sb = sb.tile([C, NPAIR, 2, C], FP8, name="w2")
        nc.sync.dma_start(out=w2_sb, in_=w2_d[:, :, :, :])
        w3_sb = sb.tile([C, NPAIR, 2, C], FP8, name="w3")
        nc.sync.dma_start(out=w3_sb, in_=w3_d[:, :, :, :])
        bn_sb = {}
        for key, dt_ in bn_d.items():
            bn_sb[key] = sb.tile([C, 1], F32, name=f"bn{key[1]}{key[0]}")
            nc.sync.dma_start(out=bn_sb[key], in_=dt_[:, :])
        fcw_sb = sb.tile([C, 10], F32)
        nc.sync.dma_start(out=fcw_sb, in_=fcw_d[:, :])
        fcb_sb = sb.tile([1, 10], F32)
        nc.sync.dma_start(out=fcb_sb, in_=fcb_d[:, :])
        eps_t = {}
        for l, ev in ((1, BN_EPS), (2, 4 * BN_EPS), (3, 4 * BN_EPS)):
            eps_t[l] = sb.tile([C, 1], F32, name=f"eps{l}")
            nc.vector.memset(eps_t[l], ev)

        # padded input tiles (input arrives host-padded, borders zero)
        xpad = [sb.tile([C, XPADN], F32R, name=f"xpad{i}")
                for i in range(NXBUF)]
        xpad3 = [t_.rearrange("c (h w) -> c h w", h=34) for t_ in xpad]

        # padded sign-spike tiles, borders stay -1 (= spike 0)
        spk = [sb.tile([C, SPADN], FP8, name=f"spk{i}") for i in range(8)]
        for t_ in spk:
            nc.vector.memset(t_, -1.0)
        spk3 = [t_.rearrange("c (h w) -> c h w", h=SHPS) for t_ in spk]

        # rolling f32 y slots
        ybuf = sb.tile([C, NSLOT, HW], F32, name="y")

        # LIF state
        u = sb.tile([C, BL * HW], F32, name="u")
        u4 = u.rearrange("c (b p) -> c b p", b=BL)
        gate_b = [sb.tile([C, HW], FP16, name=f"gate{i}") for i in range(2)]
        sgn_s = sb.tile([C, HW], FP16, name="sgn")
        poolbuf = sb.tile([C, T, BL], F32)

        nstat = T * BL * NHALF // STATS_STRIDE
        stats = {l: sb.tile([C, nstat, 6], F32, name=f"stats{l}")
                 for l in (1, 2, 3)}

        # ---------- helpers ----------
        def _xwin(src3, r0, dy, dx):
            return src3[:, r0 + dy:r0 + dy + RH, dx:dx + W]

        def _swin_pair(src3, r0, dy, dx):
            wv = src3[:, r0 + dy:r0 + dy + RH, dx:dx + W].copy()
            wv.ap = [wv.ap[0], [SWP, 2]] + wv.ap[1:]
            return wv

        def conv_t(l, t, srcs):
            """Emit conv of layer l for all 8 (b, half) tiles of timestep t.
            srcs: 4 padded views (per b)."""
            pts = []
            for b in range(BL):
                for h in range(NHALF):
                    pt = psum_pool.tile([C, RH * W], F32, name=f"ps{b}{h}",
                                        tag=f"ps{b}{h}")
                    pts.append(pt)
            if l == 1:
                for ik, (dy, dx) in enumerate(TAPS):
                    lhsT = w1_sb[:, ik, :]
                    for j in range(8):
                        b, h = divmod(j, NHALF)
                        nc.tensor.matmul(
                            pts[j], lhsT, _xwin(srcs[b], h * RH, dy, dx),
                            start=(ik == 0), stop=(ik == len(TAPS) - 1),
                            skip_group_check=True)
            else:
                w_sb = w2_sb if l == 2 else w3_sb
                for ip in range(NPAIR):
                    (dyA, dxA), _ = GPAIRS[ip % len(GPAIRS)]
                    lhsT = w_sb[:, ip]
                    for j in range(8):
                        b, h = divmod(j, NHALF)
                        nc.tensor.matmul(
                            pts[j], lhsT, _swin_pair(srcs[b], h * RH,
                                                     dyA, dxA),
                            start=(ip == 0), stop=(ip == NPAIR - 1),
                            perf_mode=DR, skip_group_check=True)
            for j in range(8):
                b, h = divmod(j, NHALF)
                img = t * BL + b
                slot = (YBASE[l] + img) % NSLOT
                if j % STATS_STRIDE == 0:
                    nc.vector.bn_stats(
                        out=stats[l][:, (t * 8 + j) // STATS_STRIDE, :],
                        in_=pts[j])
                nc.scalar.copy(
                    out=ybuf[:, slot, h * RH * W:(h + 1) * RH * W],
                    in_=pts[j])

        def layer_stats(l, rep=0):
            """bn_aggr + AllReduce -> (dp, thp, negthp) [C,1] tiles."""
            sx = f"{l}_{rep}"
            mv = sb.tile([C, 2], F32, name=f"mv{sx}")
            nc.vector.bn_aggr(out=mv, in_=stats[l])
            cc = sb.tile([C, 2], F32, name=f"cc{sx}")
            nc.gpsimd.tensor_copy(out=cc[:, 0:1], in_=mv[:, 0:1])
            sq = sb.tile([C, 1], F32, name=f"sq{sx}")
            nc.vector.tensor_tensor(sq, mv[:, 0:1], mv[:, 0:1], op=ALU.mult)
            nc.vector.tensor_tensor(cc[:, 1:2], mv[:, 1:2], sq, op=ALU.add)
            cc_in, cc_out = cc_bufs[l]
            nc.sync.dma_start(out=cc_in[:, :], in_=cc)
            if not SIM1:
                nc.gpsimd.collective_compute(
                    "AllReduce", ALU.add,
                    replica_groups=[list(range(NCORES))],
                    ins=[cc_in[:, :]], outs=[cc_out[:, :]],
                )
            ccr = sb.tile([C, 2], F32, name=f"ccr{sx}")
            nc.sync.dma_start(out=ccr,
                              in_=cc_in[:, :] if SIM1 else cc_out[:, :])
            nshard = 1 if SIM1 else NCORES
            mean = sb.tile([C, 1], F32, name=f"mean{sx}")
            nc.vector.tensor_scalar(mean, ccr[:, 0:1], 1.0 / nshard, None,
                                    op0=ALU.mult)
            ex2 = sb.tile([C, 1], F32, name=f"ex2{sx}")
            nc.vector.tensor_scalar(ex2, ccr[:, 1:2], 1.0 / nshard, None,
                                    op0=ALU.mult)
            var = sb.tile([C, 1], F32, name=f"var{sx}")
            nc.vector.tensor_tensor(var, mean, mean, op=ALU.mult)
            nc.vector.tensor_tensor(var, ex2, var, op=ALU.subtract)
            sd = sb.tile([C, 1], F32, name=f"sd{sx}")
            nc.scalar.activation(sd, var, AF.Sqrt, bias=eps_t[l], scale=1.0)
            ginv = sb.tile([C, 1], F32, name=f"ginv{sx}")
            nc.vector.reciprocal(out=ginv, in_=bn_sb[(l, "w")])
            tmp = sb.tile([C, 1], F32, name=f"tmp{sx}")
            nc.vector.tensor_tensor(tmp, sd, ginv, op=ALU.mult)
            thp = sb.tile([C, 1], F32, name=f"thp{sx}")
            nc.vector.tensor_scalar(thp, tmp, THRESH, None, op0=ALU.mult)
            negthp = sb.tile([C, 1], F32, name=f"negthp{sx}")
            nc.vector.tensor_scalar(negthp, tmp, -THRESH, None, op0=ALU.mult)
            dp = sb.tile([C, 1], F32, name=f"dp{sx}")
            nc.vector.tensor_tensor(dp, bn_sb[(l, "b")], tmp, op=ALU.mult)
            nc.vector.tensor_tensor(dp, dp, mean, op=ALU.subtract)
            if DEBUG:
                nc.sync.dma_start(out=dbg[f"ccr{l}"][:, :], in_=ccr)
                nc.sync.dma_start(out=dbg[f"dp{l}"][:, :], in_=dp)
                nc.sync.dma_start(out=dbg[f"thp{l}"][:, :], in_=thp)
                nc.sync.dma_start(out=dbg[f"y{l}"][:, :],
                                  in_=ybuf[:, YBASE[l] % NSLOT, :])
            return dp, thp, negthp

        def lif_step(l, t, b, dp, thp, negthp):
            img = t * BL + b
            yv = ybuf[:, (YBASE[l] + img) % NSLOT, :]
            ub = u4[:, b]
            if t == 0:
                nc.scalar.activation(ub, yv, AF.Identity, bias=dp, scale=1.0)
            else:
                g = gate_b[(t * BL + b) % 2]
                nc.gpsimd.tensor_scalar(g, ub, thp, DECAY,
                                        op0=ALU.is_le, op1=ALU.mult)
                nc.vector.tensor_tensor(ub, ub, g, op=ALU.mult)
                nc.vector.scalar_tensor_tensor(ub, yv, dp, ub,
                                               op0=ALU.add, op1=ALU.add)
            if l < 3:
                dest = spk3[img % 8][:, 1:1 + H, 1:1 + W]
                nc.scalar.activation(
                    dest, ub.rearrange("c (h w) -> c h w", h=H),
                    AF.Sign, bias=negthp, scale=1.0)
                if DEBUG and img == 0:
                    nc.sync.dma_start(out=dbg[f"spk{l}"][:, :],
                                      in_=spk[0])
            else:
                nc.scalar.activation(sgn_s, ub, AF.Sign, bias=negthp,
                                     scale=1.0,
                                     accum_out=poolbuf[:, t, b:b + 1])

        for rep in range(REPS):
            # ---------- chunk 1: input DMA + conv1 ----------
            for t in range(T):
                for b in range(BL):
                    img = t * BL + b
                    nc.sync.dma_start(
                        out=xpad[img % NXBUF],
                        in_=x_d[t, b, :, :].bitcast(F32R))
                conv_t(1, t, [xpad3[(t * BL + b) % NXBUF]
                              for b in range(BL)])
            dp1, thp1, negthp1 = layer_stats(1, rep)

            # ---------- chunk 2: LIF1 + conv2 ----------
            for t in range(T):
                for b in range(BL):
                    lif_step(1, t, b, dp1, thp1, negthp1)
                conv_t(2, t, [spk3[(t * BL + b) % 8] for b in range(BL)])
            dp2, thp2, negthp2 = layer_stats(2, rep)

            # ---------- chunk 3: LIF2 + conv3 ----------
            for t in range(T):
                for b in range(BL):
                    lif_step(2, t, b, dp2, thp2, negthp2)
                conv_t(3, t, [spk3[(t * BL + b) % 8] for b in range(BL)])
            dp3, thp3, negthp3 = layer_stats(3, rep)

            # ---------- chunk 4: LIF3 -> poolbuf ----------
            for t in range(T):
                for b in range(BL):
                    lif_step(3, t, b, dp3, thp3, negthp3)

            if DEBUG:
                nc.sync.dma_start(out=dbg["pool"][:, :],
                                  in_=poolbuf.rearrange("c t b -> c (t b)"))

            # ---------- head: pooling + FC ----------
            feat = sb.tile([C, BL], F32, name=f"feat{rep}")
            for b in range(BL):
                nc.vector.tensor_reduce(feat[:, b:b + 1], poolbuf[:, :, b],
                                        axis=mybir.AxisListType.X,
                                        op=ALU.add)
            # sgn sum -> spike count: count = 0.5*sum + T*HW/2
            nc.vector.tensor_scalar(feat, feat, 0.5, T * HW / 2.0,
                                    op0=ALU.mult, op1=ALU.add)
            prod = sb.tile([C, BL, 10], F32, name=f"prod{rep}")
            nc.vector.tensor_tensor(
                prod, feat.unsqueeze(2).broadcast_to([C, BL, 10]),
                fcw_sb.unsqueeze(1).broadcast_to([C, BL, 10]), op=ALU.mult)
            red = sb.tile([C, BL, 10], F32, name=f"red{rep}")
            nc.gpsimd.partition_all_reduce(red, prod, channels=C,
                                           reduce_op=bass_isa.ReduceOp.add)
            ofin = sb.tile([1, BL, 10], F32, name=f"ofin{rep}")
            nc.vector.tensor_scalar(ofin, red[0:1], 1.0 / POOL_N, None,
                                    op0=ALU.mult)
            nc.vector.tensor_tensor(
                ofin, ofin, fcb_sb.unsqueeze(1).broadcast_to([1, BL, 10]),
                op=ALU.add)
            nc.sync.dma_start(out=out_d[:, :],
                              in_=ofin.rearrange("c b k -> c (b k)"))

    nc.compile()
    return nc


_NC_CACHE = {}


def _get_nc():
    if "nc" not in _NC_CACHE:
        _NC_CACHE["nc"] = build()
    return _NC_CACHE["nc"]


def _pack_fp8_pairs(wt):
    """[C,9,C] f32 -> [C,12,2,C] fp8e4: 6 geometric pairs x (hi, lo),
    weights pre-scaled by W8SCALE (dummy taps zero)."""
    w = wt.astype(np.float32) * W8SCALE
    hi = w.astype(ml_dtypes.float8_e4m3fn)
    lo = (w - hi.astype(np.float32)).astype(ml_dtypes.float8_e4m3fn)
    out = np.zeros((C, NPAIR, 2, C), dtype=ml_dtypes.float8_e4m3fn)
    for g, ((dyA, dxA), (dyB, dxB)) in enumerate(GPAIRS):
        for term, src in ((0, hi), (1, lo)):
            ip = term * len(GPAIRS) + g
            out[:, ip, 0, :] = src[:, dyA * 3 + dxA, :]
            if dyB < 3:
                out[:, ip, 1, :] = src[:, dyB * 3 + dxB, :]
    return out


def make_in_maps(inp, conv_ws, bns, fc_w, fc_b):
    """Build the 8 per-core input maps from full (numpy) model inputs."""
    common = {}
    wts = [np.ascontiguousarray(w.transpose(1, 2, 3, 0).reshape(C, 9, C))
           for w in conv_ws]  # [I, k, O]
    common["w1"] = wts[0]
    common["w2"] = _pack_fp8_pairs(wts[1])
    common["w3"] = _pack_fp8_pairs(wts[2])
    for li in (1, 2, 3):
        common[f"bnw{li}"] = np.ascontiguousarray(
            bns[li - 1][0].reshape(C, 1))
        common[f"bnb{li}"] = np.ascontiguousarray(
            bns[li - 1][1].reshape(C, 1))
    common["fcw"] = np.ascontiguousarray(fc_w.T)          # [C, 10]
    common["fcb"] = np.ascontiguousarray(fc_b.reshape(1, 10))

    in_maps = []
    for cid in range(NCORES):
        xc = inp[:, cid * BL:(cid + 1) * BL]      # [T, BL, C, H, W]
        xp = np.zeros((T, BL, C, 34, XWP), np.float32)
        xp[:, :, :, 1:1 + H, 1:1 + W] = xc
        m = dict(common)
        m["x"] = np.ascontiguousarray(xp.reshape(T, BL, C, XPADN))
        in_maps.append(m)
    return in_maps


def kernel(inp, conv_w1, conv_w2, conv_w3, bn_w1, bn_b1, bn_w2, bn_b2,
           bn_w3, bn_b3, fc_w, fc_b):
    inp = np.asarray(inp, dtype=np.float32)
    ws = [np.asarray(w, dtype=np.float32) for w in (conv_w1, conv_w2, conv_w3)]
    bns = [(np.asarray(bn_w1, np.float32), np.asarray(bn_b1, np.float32)),
           (np.asarray(bn_w2, np.float32), np.asarray(bn_b2, np.float32)),
           (np.asarray(bn_w3, np.float32), np.asarray(bn_b3, np.float32))]
    fc_w = np.asarray(fc_w, np.float32)
    fc_b = np.asarray(fc_b, np.float32)

    nc = _get_nc()
    in_maps = make_in_maps(inp, ws, bns, fc_w, fc_b)
    res = run_bass_kernel_spmd(nc, in_maps, core_ids=list(range(NCORES)))
    out = np.concatenate(
        [r["out"].reshape(BL, 10) for r in res.results], axis=0)
    return out.astype(np.float32)
